# revision 1
# baseline (speedup 1.0000x reference)
"""Bias multi-head attention kernel for Trainium2 (8 NeuronCores).

Problem: x[B=4,N=2048,D=1024], 16 heads, dh=64; attn bias (scaled by
beta) added to the first 8 heads; qkv proj -> attention -> out proj.

Sharding: core = (b, parity); b = core//2, parity = core%2. Each core
handles batch b and the 8 heads hlist = [parity, parity+2, ...] (the 4
biased heads h<8 plus 4 unbiased). Each core computes the partial
y_b = sum over its heads; the host adds the two parity cores per batch.

Per-core device pipeline (all matmuls float32r — full-rate fp32):
  phase 1: k^T head-pairs and V (with an appended ones column) are made
           SBUF-resident; q^T (pre-scaled by 1/sqrt(dh)) spills to DRAM.
  phase 2: per n-block and head pair: S^T = K^T.T @ q^T per m-tile (the
           two heads run concurrently in PE row-groups 0-63/64-127 into
           one 2-bank PSUM tile), + beta*bias^T (DVE), one exp (ACT),
           then P^T @ [V|1] accumulates over m (PSUM rows 0-63 = o^T,
           row 64 = softmax denominator). 1/denominator (DVE) is
           broadcast across partitions via a DRAM-bounce DMA (the only
           partition-broadcast this toolchain accepts), o^T is scaled,
           and the output projection accumulates head pairs (K=128).
"""

import numpy as np
import concourse.bass as bass
import concourse.mybir as mybir
import concourse.tile as tile

f32 = mybir.dt.float32
f32r = mybir.dt.float32r
AF = mybir.ActivationFunctionType
ALU = mybir.AluOpType

N = 2048
D = 1024
NB = 512
NBLK = N // NB
MT = N // 128
KD = D // 128
SCALE = 0.125


def _split_multi_waits(nc, limit=1):
    """This walrus build rejects >1 sync wait per instruction; hoist
    extra waits onto standalone same-engine NoOps placed before it."""
    n_split = 0
    for f in nc.m.functions:
        for bb in f.blocks:
            new_insts = []
            for inst in bb.instructions:
                si = inst.sync_info
                waits = list(si.on_wait) if si is not None and si.on_wait else []
                if len(waits) > limit:
                    extra, keep = waits[:-limit], waits[-limit:]
                    for i in range(0, len(extra), limit):
                        nop = mybir.InstNoOp(
                            name=f"{inst.name}.wsplit{i}", ins=[], outs=[]
                        )
                        nop.engine = inst.engine
                        nop.sync_info = mybir.SyncInfo(
                            on_wait=extra[i : i + limit], on_update=[]
                        )
                        nc.register_instruction(nop, overwrite=True)
                        new_insts.append(nop)
                        n_split += 1
                    inst.sync_info = mybir.SyncInfo(
                        on_wait=keep, on_update=list(si.on_update or [])
                    )
                new_insts.append(inst)
            bb.instructions = new_insts
    return n_split


def build_nc(reps=1, biased_heads=4):
    nc = bass.Bass("TRN2", debug=False)
    xT = nc.dram_tensor("xT", [D, N], f32r, kind="ExternalInput")
    wqkvT = nc.dram_tensor("wqkvT", [D, 1536], f32r, kind="ExternalInput")
    wprojT = nc.dram_tensor("wprojT", [512, 1024], f32r, kind="ExternalInput")
    biasT = nc.dram_tensor("biasT", [N, N], f32, kind="ExternalInput")
    beta4 = nc.dram_tensor("beta4", [1, 4], f32r, kind="ExternalInput")
    qspill = nc.dram_tensor("qspill", [512, N], f32r)
    rspill = nc.dram_tensor("rspill", [8, NB], f32)
    y = nc.dram_tensor("y", [N, D], f32, kind="ExternalOutput")

    with tile.TileContext(nc) as tc:
        with (
            tc.tile_pool(name="const", bufs=1) as const_pool,
            tc.tile_pool(name="kvres", bufs=1) as kvres,
            tc.tile_pool(name="wproj", bufs=1) as wproj_pool,
        ):
            ones_row = const_pool.tile([1, 128], f32r, tag="ones")
            nc.vector.memset(ones_row[:].bitcast(f32), 1.0)
            beta_row = const_pool.tile([1, 4], f32r, tag="betar")
            nc.sync.dma_start(out=beta_row[:], in_=beta4[:])
            beta_cols = const_pool.tile([128, 4], f32, tag="betac")

            kT = [kvres.tile([128, N], f32r, tag=f"kT{j}", name=f"kT{j}") for j in range(4)]
            V = [kvres.tile([128, 8 * 65], f32r, tag=f"V{m}", name=f"V{m}") for m in range(MT)]
            wproj_sb = [
                wproj_pool.tile([128, 1024], f32r, tag=f"wp{j}", name=f"wp{j}")
                for j in range(4)
            ]
            for j in range(4):
                nc.sync.dma_start(
                    out=wproj_sb[j][:], in_=wprojT[j * 128 : (j + 1) * 128, :]
                )
            for m in range(MT):
                nc.vector.memset(
                    V[m][:].rearrange("p (h c) -> p h c", c=65)[:, :, 64:65].bitcast(f32),
                    1.0,
                )

            def body(_=None):
                with tc.tile_pool(name="beta_ps", bufs=1, space="PSUM") as bps:
                    bcast = bps.tile([128, 4], f32, tag="betaps")
                    nc.tensor.matmul(
                        bcast[:], ones_row[:], beta_row[:], start=True, stop=True
                    )
                    nc.scalar.copy(beta_cols[:], bcast[:])

                # phase 1: QKV
                with (
                    tc.tile_pool(name="wqkv", bufs=KD) as wpool,
                    tc.tile_pool(name="xs", bufs=10) as xpool,
                    tc.tile_pool(name="qk_ps", bufs=3, space="PSUM") as qk_ps,
                    tc.tile_pool(name="v_ps", bufs=2, space="PSUM") as v_ps,
                    tc.tile_pool(name="qtmp", bufs=3) as qtmp_pool,
                ):
                    w_sb = []
                    for k in range(KD):
                        wt = wpool.tile([128, 1536], f32r, tag="w")
                        nc.sync.dma_start(
                            out=wt[:], in_=wqkvT[k * 128 : (k + 1) * 128, :]
                        )
                        w_sb.append(wt)
                    for nb in range(NBLK):
                        x_sb = []
                        for k in range(KD):
                            xt = xpool.tile([128, NB], f32r, tag="x")
                            nc.sync.dma_start(
                                out=xt[:],
                                in_=xT[k * 128 : (k + 1) * 128, nb * NB : (nb + 1) * NB],
                            )
                            x_sb.append(xt)
                        for e in range(8):
                            ps = qk_ps.tile([128, NB], f32, tag="qk")
                            for k in range(KD):
                                nc.tensor.matmul(
                                    ps[:],
                                    w_sb[k][:, e * 128 : (e + 1) * 128],
                                    x_sb[k][:],
                                    start=(k == 0),
                                    stop=(k == KD - 1),
                                )
                            if e < 4:
                                qt = qtmp_pool.tile([128, NB], f32r, tag="qt")
                                nc.scalar.mul(qt[:], ps[:], SCALE)
                                nc.sync.dma_start(
                                    out=qspill[
                                        e * 128 : (e + 1) * 128, nb * NB : (nb + 1) * NB
                                    ],
                                    in_=qt[:],
                                )
                            else:
                                j = e - 4
                                nc.scalar.copy(kT[j][:, nb * NB : (nb + 1) * NB], ps[:])
                        for mi in range(4):
                            m = nb * 4 + mi
                            ps = v_ps.tile([128, NB], f32, tag="v")
                            for k in range(KD):
                                nc.tensor.matmul(
                                    ps[:],
                                    x_sb[k][:, mi * 128 : (mi + 1) * 128],
                                    w_sb[k][:, 1024:1536],
                                    start=(k == 0),
                                    stop=(k == KD - 1),
                                )
                            nc.scalar.copy(
                                V[m][:].rearrange("p (h c) -> p h c", c=65)[:, :, 0:64],
                                ps[:].rearrange("p (h c) -> p h c", c=64),
                            )

                # phase 2: attention + projection
                with (
                    tc.tile_pool(name="qblk", bufs=8) as qblk_pool,
                    tc.tile_pool(name="bias", bufs=18) as bias_pool,
                    tc.tile_pool(name="esb", bufs=6) as e_pool,
                    tc.tile_pool(name="opair", bufs=8) as o_pool,
                    tc.tile_pool(name="ysb", bufs=3) as y_pool,
                    tc.tile_pool(name="nrm", bufs=6) as nrm_pool,
                    tc.tile_pool(name="l_ps", bufs=2, space="PSUM") as l_ps,
                    tc.tile_pool(name="o_ps", bufs=3, space="PSUM") as o_psp,
                    tc.tile_pool(name="y_ps", bufs=1, space="PSUM") as y_psp,
                ):
                    for nb in range(NBLK):
                        qblk = []
                        for j in range(4):
                            qt = qblk_pool.tile([128, NB], f32r, tag="qb")
                            nc.sync.dma_start(
                                out=qt[:],
                                in_=qspill[
                                    j * 128 : (j + 1) * 128, nb * NB : (nb + 1) * NB
                                ],
                            )
                            qblk.append(qt)
                        bias_sb = []
                        for m in range(MT):
                            bt = bias_pool.tile([128, NB], f32, tag="bias")
                            nc.sync.dma_start(
                                out=bt[:],
                                in_=biasT[
                                    m * 128 : (m + 1) * 128, nb * NB : (nb + 1) * NB
                                ],
                            )
                            bias_sb.append(bt)

                        o_pairs = []
                        for j in range(4):
                            o_pair = o_pool.tile([128, NB], f32r, tag="op")
                            o_pairs.append(o_pair)
                            o_ps_pair = [
                                o_psp.tile([65, NB], f32, tag="ops", name=f"ops{j}_{hh}")
                                for hh in range(2)
                            ]
                            for m in range(MT):
                                lp = l_ps.tile([128, 2 * NB], f32, tag="lp")
                                for hh in range(2):
                                    off = hh * 64
                                    nc.tensor.matmul(
                                        lp[:, hh * NB : (hh + 1) * NB],
                                        kT[j][off : off + 64, m * 128 : (m + 1) * 128],
                                        qblk[j][off : off + 64, :],
                                        start=True,
                                        stop=True,
                                        tile_position=(off, 0),
                                    )
                                for hh in range(2):
                                    h_idx = 2 * j + hh
                                    if h_idx < biased_heads:
                                        nc.vector.scalar_tensor_tensor(
                                            lp[:, hh * NB : (hh + 1) * NB],
                                            bias_sb[m][:],
                                            beta_cols[:, h_idx : h_idx + 1],
                                            lp[:, hh * NB : (hh + 1) * NB],
                                            op0=ALU.mult,
                                            op1=ALU.add,
                                        )
                                et = e_pool.tile([128, 2 * NB], f32r, tag="e")
                                nc.scalar.activation(et[:], lp[:], AF.Exp)
                                for hh in range(2):
                                    h = 2 * j + hh
                                    nc.tensor.matmul(
                                        o_ps_pair[hh][:],
                                        V[m][:, h * 65 : (h + 1) * 65],
                                        et[:, hh * NB : (hh + 1) * NB],
                                        start=(m == 0),
                                        stop=(m == MT - 1),
                                    )
                            for hh in range(2):
                                off = hh * 64
                                recip = nrm_pool.tile([1, NB], f32, tag="recip")
                                nc.vector.reciprocal(
                                    recip[:], o_ps_pair[hh][64:65, :]
                                )
                                ridx = 2 * j + hh
                                nc.sync.dma_start(
                                    out=rspill[ridx : ridx + 1, :], in_=recip[:]
                                )
                                bcs = nrm_pool.tile([64, NB], f32, tag="bcs")
                                nc.sync.dma_start(
                                    out=bcs[:],
                                    in_=rspill[ridx : ridx + 1, :].to_broadcast(
                                        [64, NB]
                                    ),
                                )
                                nc.vector.tensor_tensor(
                                    o_pair[off : off + 64, :],
                                    o_ps_pair[hh][0:64, :],
                                    bcs[:],
                                    ALU.mult,
                                )
                        for nt in range(4):
                            for db in range(2):
                                yp = y_psp.tile([128, 512], f32, tag="yp")
                                for j in range(4):
                                    nc.tensor.matmul(
                                        yp[:],
                                        o_pairs[j][:, nt * 128 : (nt + 1) * 128],
                                        wproj_sb[j][:, db * 512 : (db + 1) * 512],
                                        start=(j == 0),
                                        stop=(j == 3),
                                    )
                                ysb = y_pool.tile([128, 512], f32, tag="y")
                                nc.vector.tensor_copy(ysb[:], yp[:])
                                nc.sync.dma_start(
                                    out=y[
                                        nb * NB + nt * 128 : nb * NB + (nt + 1) * 128,
                                        db * 512 : (db + 1) * 512,
                                    ],
                                    in_=ysb[:],
                                )

            if reps == 1:
                body()
            else:
                with tc.For_i(0, reps, 1):
                    body()

    _split_multi_waits(nc)
    nc.finalize()
    return nc


def make_core_inputs(x, attn_bias, Wqkv, Wproj, beta, core_id):
    b = core_id // 2
    parity = core_id % 2
    hlist = list(range(parity, 16, 2))
    rows = np.concatenate([np.arange(h * 64, (h + 1) * 64) for h in hlist])
    wqkvT = np.ascontiguousarray(
        np.concatenate([Wqkv[rows], Wqkv[D + rows], Wqkv[2 * D + rows]], 0).T
    )
    wprojT = np.ascontiguousarray(Wproj.T[rows])
    beta4 = (
        np.asarray(beta).reshape(-1)[hlist[:4]].reshape(1, 4).astype(np.float32)
    )
    return {
        "xT": np.ascontiguousarray(x[b].T).astype(np.float32),
        "wqkvT": wqkvT.astype(np.float32),
        "wprojT": wprojT.astype(np.float32),
        "biasT": np.ascontiguousarray(attn_bias[b, 0].T).astype(np.float32),
        "beta4": beta4,
    }


_NC_CACHE = {}


def kernel(x, attn_bias, Wqkv, Wproj, beta):
    from concourse.bass_utils import run_bass_kernel_spmd

    x = np.asarray(x, dtype=np.float32)
    attn_bias = np.asarray(attn_bias, dtype=np.float32)
    Wqkv = np.asarray(Wqkv, dtype=np.float32)
    Wproj = np.asarray(Wproj, dtype=np.float32)
    beta = np.asarray(beta, dtype=np.float32)

    if "nc" not in _NC_CACHE:
        _NC_CACHE["nc"] = build_nc(reps=1)
    nc = _NC_CACHE["nc"]

    in_maps = [
        make_core_inputs(x, attn_bias, Wqkv, Wproj, beta, core) for core in range(8)
    ]
    res = run_bass_kernel_spmd(nc, in_maps, core_ids=list(range(8)))
    out = np.zeros((4, N, D), dtype=np.float32)
    for core_id in range(8):
        out[core_id // 2] += res.results[core_id]["y"]
    return out

